# revision 1
# baseline (speedup 1.0000x reference)
import numpy as np
import jax
import jax.numpy as jnp

# nn_CustomLoss grid-path loss, data-parallel over 8 NeuronCores.
# Full inputs in, full output out; batch dim sharded 8 ways inside.

WEIGHT = 1000.0
LONELY_W = 15.0
CLUSTER_W = 5.0
CELL_W = 20.0
CSIZE_W = 12.0
GAP_W = 300.0
H = W = 10
N_CORES = 8


def _md(y1, x1, y2, x2):
    return jnp.abs(y1 - y2) + jnp.abs(x1 - x2)


def _per_sample(result, points):
    # result: [H, W] float32, points: [2, 2] int32
    p0y, p0x = points[0, 0], points[0, 1]
    p1y, p1x = points[1, 0], points[1, 1]
    base = _md(p0y, p0x, p1y, p1x)

    rows = jnp.arange(H)[:, None]
    cols = jnp.arange(W)[None, :]

    # one-hot reads instead of dynamic gathers (device-friendlier, exact:
    # products are 0 or the value itself, sum over zeros is exact)
    oh0 = ((rows == p0y) & (cols == p0x)).astype(result.dtype)
    oh1 = ((rows == p1y) & (cols == p1x)).astype(result.dtype)
    r0 = jnp.sum(result * oh0)
    r1 = jnp.sum(result * oh1)
    loss_start = (2.0 - (r0 + r1)) * WEIGHT

    mask = jnp.round(result) == 1.0
    seed = (rows == p0y) & (cols == p0x)

    def dilate(_, c):
        p = jnp.pad(c, 1)
        nb = (p[:-2, :-2] | p[:-2, 1:-1] | p[:-2, 2:] |
              p[1:-1, :-2] | p[1:-1, 2:] |
              p[2:, :-2] | p[2:, 1:-1] | p[2:, 2:])
        return c | (nb & mask)

    cluster = jax.lax.fori_loop(0, H + W, dilate, seed)

    d2 = _md(rows, cols, p0y, p0x) + _md(rows, cols, p1y, p1x)
    delta = (d2 - base).astype(result.dtype)
    single_cell = jnp.sum(jnp.where(delta == 0.0,
                                    (1.0 - result) * CELL_W,
                                    result * delta * 0.5))

    in_cl = cluster.T
    csize = jnp.sum(cluster).astype(result.dtype)
    lonelyness = jnp.sum(jnp.where(in_cl, (1.0 - result) * CLUSTER_W,
                                   result * LONELY_W))
    cluster_size_pen = jnp.sum(jnp.where(in_cl, result * csize * CSIZE_W, 0.0))

    dist_end = _md(rows, cols, p1y, p1x)
    masked = jnp.where(cluster, dist_end, H * W + 10).reshape(-1)
    idx = jnp.argmin(masked)
    minval = masked[idx]
    better = minval < base
    ny = jnp.where(better, idx // W, p0y).astype(jnp.int32)
    nx = jnp.where(better, idx % W, p0x).astype(jnp.int32)
    gap = jnp.minimum(base, minval)

    oy = p1y - ny
    ox = p1x - nx

    def upd(cond, cy, cx, st):
        by, bx, bg = st
        d = _md(cy, cx, p1y, p1x)
        b = cond & (d < bg)
        return (jnp.where(b, cy, by), jnp.where(b, cx, bx), jnp.where(b, d, bg))

    st = (ny, nx, gap)
    c = ox < 0
    st = upd(c, ny, nx - 1, st)
    st = upd(c & (ny != 0), ny - 1, nx - 1, st)
    st = upd(c & (ny != H - 1), ny + 1, nx - 1, st)
    c = ox > 0
    st = upd(c, ny, nx + 1, st)
    st = upd(c & (ny != 0), ny - 1, nx + 1, st)
    st = upd(c & (ny != H - 1), ny + 1, nx + 1, st)
    st = upd(oy < 0, ny - 1, nx, st)
    st = upd(oy > 0, ny + 1, nx, st)
    ncy = jnp.clip(st[0], 0, H - 1)
    ncx = jnp.clip(st[1], 0, W - 1)

    ohn = ((rows == ncy) & (cols == ncx)).astype(result.dtype)
    rn = jnp.sum(result * ohn)
    gap_pen = gap.astype(result.dtype) * GAP_W * (1.0 - rn)

    return loss_start + lonelyness + single_cell + cluster_size_pen + gap_pen


def _shard_loss(result_shard, points_shard):
    losses = jax.vmap(_per_sample)(result_shard[:, 0], points_shard)
    return jnp.sum(losses)


_pmapped = jax.pmap(_shard_loss)
_cpu_jit = None


def _cpu_fallback(result_given, points_given):
    global _cpu_jit
    cpu = jax.devices("cpu")[0]
    if _cpu_jit is None:
        _cpu_jit = jax.jit(_shard_loss, device=cpu)
    B = result_given.shape[0]
    total = np.float32(0.0)
    chunk = 32768
    for s in range(0, B, chunk):
        r = jax.device_put(result_given[s:s + chunk], cpu)
        p = jax.device_put(points_given[s:s + chunk], cpu)
        total += np.float32(_cpu_jit(r, p))
    return np.asarray(total, dtype=np.float32).reshape(1)


def kernel(result_given, points_given):
    result_given = np.asarray(result_given, dtype=np.float32)
    points_given = np.asarray(points_given)
    if points_given.dtype != np.int32:
        points_given = points_given.astype(np.int32)
    B = result_given.shape[0]
    if B % N_CORES != 0:
        return _cpu_fallback(result_given, points_given)
    try:
        rs = result_given.reshape(N_CORES, B // N_CORES, 1, H, W)
        ps = points_given.reshape(N_CORES, B // N_CORES, 2, 2)
        partials = np.asarray(_pmapped(rs, ps), dtype=np.float32)
        return np.sum(partials, dtype=np.float32).reshape(1).astype(np.float32)
    except Exception:
        return _cpu_fallback(result_given, points_given)
